# revision 1
# baseline (speedup 1.0000x reference)
"""Trainium2 Bass kernel for nn_MCQuantiles (ThreeCompNode SNN scan).

Strategy (8 NeuronCores, data-parallel over batch):
- Each core takes 8 batches x 32 samples = 256 rows of the B*S axis.
- Everything runs in "transposed space": feature dims on SBUF partitions,
  batch-rows on the free dim. All transposes/swizzles are done host-side for
  free; every DMA is a flat contiguous [128, X] block.
- The input matmuls (te @ Wa.T, se @ Wb.T) don't depend on the recurrence, so
  apical is computed for pairs of time steps with N=512 moving operands.
- Membrane recurrences use 2^t-scaled state so each update is a single fused
  scalar_tensor_tensor op reading the matmul result straight from PSUM:
      alpha_t = alpha_{t-1} + 2^t * apical_t         (alpha = 2^{t+1} ma)
      mu_t    = mu_{t-1} + 0.5*alpha_t + 0.5*beta_t  (mu = 2^{t+1} ms)
      spike   <=> mu > 2^{t+1}
- Layer-1 spikes are fed to the W1 matmul as q = NOT(spike) with the
  rowsum(W1)+b1 constant folded in host-side (h = c1 - q @ W1.T).
- Layer-2 spikes sp2 are fed directly to the W2 matmul; out accumulates in a
  persistent PSUM bank over all T, evicted once with scale 1/T + bias b2.
- Matmuls run in bf16 (full PE rate). Binary spike inputs are bf16-exact; the
  LIF threshold margin (|ml|max ~0.35 vs th 0.5) makes output spikes immune to
  bf16 rounding of the weights.
"""
import numpy as np
import ml_dtypes

import bass_rust
import concourse.bass as bass
import concourse.mybir as mybir
from concourse.bass_utils import run_bass_kernel_spmd
from concourse.tile import TileContext
from concourse.tile_rust import add_dep_helper

# ----- problem constants (hardcoded per contract) -----
T, B, S = 8, 64, 32
DS = DT = 3136
F = H = 512
L = 18
N_CORES = 8
NB = B // N_CORES              # 8 batches per core
R = NB * S                     # 256 rows per core
KD = 3200                      # 3136 padded to 25 k-tiles of 128
NK = KD // 128                 # 25
NPAIR = T // 2                 # 4 step pairs
NG = F // 128                  # 4 f-tiles (= h-tiles)

# column offsets inside the bf16 weight walls [128, *]
WA_COLS = NK * F               # wallA: apical weights only
O_WB = 0                       # wallM: basal weights, NK*F cols
O_SE = O_WB + NK * F           # state embeddings, NK*T*NB cols
WM_COLS = O_SE + NK * T * NB
O_W1 = 0                       # wallB: W1.T, NG*H cols
O_W2 = O_W1 + NG * H           # W2.T, NG*L cols
WB_COLS = O_W2 + NG * L

F32 = mybir.dt.float32
BF16 = mybir.dt.bfloat16
OP = mybir.AluOpType


def _patch_tile_drain():
    """This walrus build allows a single sync-wait per TPB_CTRL Drain; Tile's
    kernel-tail drain attaches one wait per active logical proc. Split them
    across a chain of drains."""
    def _patched(self, tick_clock, wait_clock):
        nc = self.nc
        drain_inst = nc.sync.drain()
        wait_clock.add_sem_waits(
            drain_inst.ins, bass_rust.ScopedClock({None: tick_clock.global_clock})
        )
        si = drain_inst.ins.sync_info
        if si is not None and len(si.on_wait) > 1:
            waits = list(si.on_wait)
            drain_inst.ins.sync_info = mybir.SyncInfo(
                on_wait=waits[:1], on_update=list(si.on_update)
            )
            for w in waits[1:]:
                extra = nc.sync.drain()
                extra.ins.sync_info = mybir.SyncInfo(on_wait=[w], on_update=[])
        nc.all_engine_barrier()
        popped = nc._tile_sem_poison_stack.pop()
        assert popped is self._sem_poison
        nc.clear_and_free_semaphores(list(self.sems.allocated().values()))
        nc.all_engine_barrier()

    TileContext._drain_and_barrier = _patched


def _split_excess_waits(nc, limit=1):
    """Walrus here rejects instructions carrying more than ~1 sync-wait. Move
    excess waits onto same-engine NoOps inserted just before the instruction."""
    for fn in nc.m.functions:
        for bb in fn.blocks:
            new = []
            changed = False
            for inst in bb.instructions:
                si = getattr(inst, "sync_info", None)
                ow = list(si.on_wait) if si is not None and si.on_wait else []
                if len(ow) > limit:
                    extra = ow[limit:]
                    for j in range(0, len(extra), limit):
                        nop = mybir.InstNoOp(
                            name=f"{inst.name}-ws{j}", ins=[], outs=[]
                        )
                        nop.engine = inst.engine
                        nop.sync_info = mybir.SyncInfo(
                            on_wait=extra[j : j + limit], on_update=[]
                        )
                        new.append(nop)
                    inst.sync_info = mybir.SyncInfo(
                        on_wait=ow[:limit], on_update=list(si.on_update)
                    )
                    changed = True
                new.append(inst)
            if changed:
                bb.set_instructions(new) if hasattr(bb, "set_instructions") else None
                if not hasattr(bb, "set_instructions"):
                    try:
                        bb.instructions[:] = new
                    except TypeError:
                        bb.instructions = new


def build_nc(with_b1=False, state_dt=BF16):  # with_b1 unused
    _patch_tile_drain()
    nc = bass.Bass()

    teT = nc.declare_dram_parameter("teT", [NPAIR, 128, NK * 2 * R], BF16, isOutput=False)
    wallA = nc.declare_dram_parameter("wallA", [128, WA_COLS], BF16, isOutput=False)
    wallM = nc.declare_dram_parameter("wallM", [128, WM_COLS], BF16, isOutput=False)
    wallB = nc.declare_dram_parameter("wallB", [128, WB_COLS], BF16, isOutput=False)
    cons = nc.declare_dram_parameter("cons", [128, NG * T + 1 + 2 * T], F32, isOutput=False)
    out = nc.declare_dram_parameter("out", [L, R], F32, isOutput=True)

    with TileContext(nc) as tc:
        with (
            tc.tile_pool(name="wpool", bufs=1) as wpool,
            tc.tile_pool(name="tepool", bufs=2) as tepool,
            tc.tile_pool(name="state", bufs=1) as state,
            tc.tile_pool(name="qpool", bufs=2) as qpool,
            tc.tile_pool(name="appool", bufs=4, space="PSUM") as appool,
            tc.tile_pool(name="hpool", bufs=3, space="PSUM") as hpool,
            tc.tile_pool(name="opool", bufs=1, space="PSUM") as opool,
        ):
            # ---- resident weights/constants ----
            CHUNKS = [2, 3, 5, 5, 5, 5]   # k-tiles per DMA chunk (25 total)
            CH_OFF = [0, 2, 5, 10, 15, 20]
            NCH = 5
            NCHUNK = len(CHUNKS)
            wallA_c = []
            prev_wa_dma = None
            for c in range(NCHUNK):
                wa_ck = wpool.tile(
                    [128, CHUNKS[c] * F], BF16, tag=f"wallA{c}", name=f"wa_ck{c}"
                )
                wallA_c.append(wa_ck)
                d = nc.sync.dma_start(
                    wa_ck[:],
                    wallA[:, CH_OFF[c] * F : (CH_OFF[c] + CHUNKS[c]) * F],
                )
                if prev_wa_dma is not None:
                    add_dep_helper(d.ins, prev_wa_dma.ins,
                                   reason="serialize wallA chunk DMAs")
                prev_wa_dma = d
            # wallM (basal+state weights) chained after wallA so basal can
            # start mid-pair-0; wallB (W1/W2) + cons right after.
            wallM_sb = wpool.tile([128, WM_COLS], BF16, tag="wallM", name="wallM_sb")
            wallB_sb = wpool.tile([128, WB_COLS], BF16, tag="wallB", name="wallB_sb")
            cons_sb = wpool.tile([128, NG * T + 1 + 2 * T], F32, tag="cons", name="cons_sb")

            def waT(k, g):
                for c in range(NCHUNK - 1, -1, -1):
                    if k >= CH_OFF[c]:
                        kk = k - CH_OFF[c]
                        return wallA_c[c][:, kk * F + g * 128 : kk * F + (g + 1) * 128]

            def wbT(k, g):
                return wallM_sb[:, O_WB + k * F + g * 128 : O_WB + k * F + (g + 1) * 128]

            def seT(k):
                return wallM_sb[:, O_SE + k * T * NB : O_SE + (k + 1) * T * NB]

            def w1T(k, g):
                return wallB_sb[:, O_W1 + k * H + g * 128 : O_W1 + k * H + (g + 1) * 128]

            def w2T(k):
                return wallB_sb[:, O_W2 + k * L : O_W2 + (k + 1) * L]

            def c1s_ap(g, t):
                return cons_sb[:, g * T + t : g * T + t + 1]

            b2_ap = cons_sb[0:L, NG * T : NG * T + 1]

            def th1_ap(t):  # -(2^{t+1})
                c = NG * T + 1 + t
                return cons_sb[:, c : c + 1]

            def th2_ap(t):  # -(2^t)
                c = NG * T + 1 + T + t
                return cons_sb[:, c : c + 1]

            # ---- state tiles ----
            A = [[state.tile([128, R], state_dt, tag=f"A{g}_{p}", name=f"A{g}_{p}")
                  for p in range(2)] for g in range(NG)]
            M = [state.tile([128, R], state_dt, tag=f"M{g}", name=f"M{g}") for g in range(NG)]
            ML = [state.tile([128, R], state_dt, tag=f"ML{g}", name=f"ML{g}") for g in range(NG)]
            Bsc = [state.tile([128, T * NB], state_dt, tag=f"Bsc{g}", name=f"Bsc{g}")
                   for g in range(NG)]

            o_psum = opool.tile([L, R], F32, tag="o", name="o_psum")

            # ---- software-pipelined main loop ----
            # Emit order interleaves pair p's recurrent chain with pair p+1's
            # apical matmul chunks so the in-order PE never head-of-line
            # blocks on spike results from the DVE.
            def emit_te_dma(pair, chain):
                tiles = []
                prev = None
                for c in range(NCHUNK):
                    tck = tepool.tile(
                        [128, CHUNKS[c] * 2 * R], BF16, tag=f"te{c}", name=f"te_ck{c}"
                    )
                    tiles.append(tck)
                    d = nc.sync.dma_start(
                        tck[:],
                        teT[pair][:, CH_OFF[c] * 2 * R
                                  : (CH_OFF[c] + CHUNKS[c]) * 2 * R],
                    )
                    if prev is not None and chain:
                        add_dep_helper(d.ins, prev.ins,
                                       reason="serialize startup te chunk DMAs")
                    prev = d
                return tiles, prev

            def emit_ap_chunk(psums, te_tiles, c):
                for g in range(NG):
                    for kk in range(CHUNKS[c]):
                        k = CH_OFF[c] + kk
                        nc.tensor.matmul(
                            psums[g][:],
                            lhsT=waT(k, g),
                            rhs=te_tiles[c][:, kk * 2 * R : (kk + 1) * 2 * R],
                            start=(k == 0),
                            stop=(k == NK - 1),
                        )

            def emit_a_updates(ap_psum, pair):
                for sub in range(2):
                    t = 2 * pair + sub
                    for g in range(NG):
                        apq = ap_psum[g][:, sub * R : (sub + 1) * R]
                        if t == 0:
                            nc.vector.tensor_scalar(
                                A[g][0][:], apq, 0.5, None, OP.mult
                            )
                        else:
                            nc.vector.scalar_tensor_tensor(
                                A[g][t % 2][:], apq, float(2 ** (t - 1)),
                                A[g][1 - t % 2][:], OP.mult, OP.add,
                            )

            def emit_basal():
                bs_psum = hpool.tile([128, T * NB], F32, tag="hq", name="bs_psum")
                for g in range(NG):
                    for k in range(NK):
                        nc.tensor.matmul(
                            bs_psum[:],
                            lhsT=wbT(k, g),
                            rhs=seT(k),
                            start=(k == 0),
                            stop=(k == NK - 1),
                        )
                    for t in range(T):
                        dst = Bsc[g][:, t * NB : (t + 1) * NB]
                        srcp = bs_psum[:, t * NB : (t + 1) * NB]
                        if t == 0:
                            nc.vector.tensor_scalar(dst, srcp, 0.5, None, OP.mult)
                        else:
                            nc.vector.scalar_tensor_tensor(
                                dst, srcp, float(2 ** (t - 1)),
                                Bsc[g][:, (t - 1) * NB : t * NB],
                                OP.mult, OP.add,
                            )

            def emit_sub(pair, sub):
                t = 2 * pair + sub
                sc_t = float(2 ** t)
                q_b16 = []
                for g in range(NG):
                    At = A[g][t % 2]
                    if t == 0:
                        nc.vector.tensor_copy(M[g][:], At[:])
                    else:
                        nc.vector.tensor_tensor(M[g][:], At[:], M[g][:], OP.add)
                    b_bc = (
                        Bsc[g][:, t * NB : (t + 1) * NB]
                        .unsqueeze(2)
                        .broadcast_to([128, NB, S])
                    )
                    m_v = M[g].rearrange("p (b s) -> p b s", s=S)
                    nc.vector.tensor_tensor(m_v, b_bc, m_v, OP.add)
                    qg = qpool.tile([128, R], BF16, tag=f"q{g}", name="qg")
                    q_b16.append(qg)
                    nc.vector.tensor_scalar(
                        qg[:], M[g][:], float(2 ** (t + 1)), None, OP.is_le
                    )
                    nc.vector.tensor_tensor(M[g][:], M[g][:], qg[:], OP.mult)

                hq_psum = []
                for g in range(NG):
                    ps = hpool.tile([128, R], F32, tag="hq", name="hq_psum")
                    hq_psum.append(ps)
                    for k in range(NG):
                        nc.tensor.matmul(
                            ps[:],
                            lhsT=w1T(k, g),
                            rhs=q_b16[k][:],
                            start=(k == 0),
                            stop=(k == NG - 1),
                        )

                sp2_b16 = []
                for g in range(NG):
                    if t == 0:
                        nc.vector.tensor_scalar(
                            ML[g][:], hq_psum[g][:], -1.0, None, OP.mult
                        )
                    else:
                        nc.vector.scalar_tensor_tensor(
                            ML[g][:], hq_psum[g][:], -sc_t, ML[g][:], OP.mult, OP.add
                        )
                    nc.scalar.activation(
                        ML[g][:], ML[g][:],
                        mybir.ActivationFunctionType.Identity,
                        bias=c1s_ap(g, t), scale=1.0,
                    )
                    spg = qpool.tile([128, R], BF16, tag=f"sp2{g}", name="spg")
                    sp2_b16.append(spg)
                    nc.vector.tensor_scalar(spg[:], ML[g][:], sc_t, None, OP.is_gt)
                    nc.vector.scalar_tensor_tensor(
                        ML[g][:], ML[g][:], sc_t, ML[g][:], OP.is_le, OP.mult
                    )

                for k in range(NG):
                    nc.tensor.matmul(
                        o_psum[:],
                        lhsT=w2T(k),
                        rhs=sp2_b16[k][:],
                        start=(t == 0 and k == 0),
                        stop=(t == T - 1 and k == NG - 1),
                    )

            # prologue: pair 0 load + apical
            te_tiles, last_te_dma = emit_te_dma(0, chain=True)
            cur_psum = [
                appool.tile([128, 2 * R], F32, tag="ap", name="ap_psum")
                for _ in range(NG)
            ]
            for c in range(NCHUNK):
                emit_ap_chunk(cur_psum, te_tiles, c)
            dM = nc.sync.dma_start(wallM_sb[:], wallM[:])
            add_dep_helper(dM.ins, last_te_dma.ins, reason="wallM after te0 chain")
            dB = nc.sync.dma_start(wallB_sb[:], wallB[:])
            add_dep_helper(dB.ins, dM.ins, reason="wallB after wallM")
            dC = nc.sync.dma_start(cons_sb[:], cons[:])
            add_dep_helper(dC.ins, dM.ins, reason="cons after wallM")
            emit_basal()

            for pair in range(NPAIR):
                emit_a_updates(cur_psum, pair)
                if pair + 1 < NPAIR:
                    te_tiles, _ = emit_te_dma(pair + 1, chain=False)
                    nxt_psum = [
                        appool.tile([128, 2 * R], F32, tag="ap", name="ap_psum")
                        for _ in range(NG)
                    ]
                    # interleave next-pair apical chunks with this pair's
                    # recurrent chain
                    emit_ap_chunk(nxt_psum, te_tiles, 0)
                    emit_ap_chunk(nxt_psum, te_tiles, 1)
                    emit_sub(pair, 0)
                    emit_ap_chunk(nxt_psum, te_tiles, 2)
                    emit_ap_chunk(nxt_psum, te_tiles, 3)
                    emit_sub(pair, 1)
                    emit_ap_chunk(nxt_psum, te_tiles, 4)
                    emit_ap_chunk(nxt_psum, te_tiles, 5)
                    cur_psum = nxt_psum
                else:
                    emit_sub(pair, 0)
                    emit_sub(pair, 1)

            # ---- final eviction: out = o_psum / T + b2 ----
            out_sb = state.tile([L, R], F32, tag="out_sb", name="out_sb")
            nc.scalar.activation(
                out_sb[:], o_psum[:],
                mybir.ActivationFunctionType.Identity,
                bias=b2_ap, scale=1.0 / T,
            )
            nc.sync.dma_start(out[:], out_sb[:])

    return nc


def _swizzle_kmaj(a, cols):
    """[KD-like rows, cols] fp -> [128, nk*cols] bf16 with [p, k*cols+c]=a[k*128+p, c]"""
    bf = ml_dtypes.bfloat16
    nk = a.shape[0] // 128
    return np.ascontiguousarray(
        a.reshape(nk, 128, cols).transpose(1, 0, 2).reshape(128, nk * cols).astype(bf)
    )


def prep_in_maps(inputs):
    """Host-side shard + transpose + pad + cast. Returns list of per-core dicts."""
    se = np.asarray(inputs["state_embedding"], np.float32)
    te = np.asarray(inputs["tau_embedding"], np.float32)
    Wb = np.asarray(inputs["Wb"], np.float32)
    Wa = np.asarray(inputs["Wa"], np.float32)
    W1 = np.asarray(inputs["W1"], np.float32)
    b1 = np.asarray(inputs["b1"], np.float32)
    W2 = np.asarray(inputs["W2"], np.float32)
    b2 = np.asarray(inputs["b2"], np.float32)
    bf = ml_dtypes.bfloat16

    def padk(a):  # pad feature axis 0 from 3136 to KD
        o = np.zeros((KD,) + a.shape[1:], a.dtype)
        o[: a.shape[0]] = a
        return o

    wallA = _swizzle_kmaj(padk(Wa.T), F)
    wallM_wb = _swizzle_kmaj(padk(Wb.T), F)
    wallB = np.empty((128, WB_COLS), bf)
    wallB[:, O_W1 : O_W1 + NG * H] = _swizzle_kmaj(np.ascontiguousarray(W1.T), H)
    wallB[:, O_W2 : O_W2 + NG * L] = _swizzle_kmaj(np.ascontiguousarray(W2.T), L)

    cons = np.zeros((128, NG * T + 1 + 2 * T), np.float32)
    c1 = W1.sum(axis=1) + b1
    for g in range(NG):
        for t in range(T):
            cons[:, g * T + t] = c1[g * 128 : (g + 1) * 128] * (2.0 ** t)
    cons[:L, NG * T] = b2
    for t in range(T):
        cons[:, NG * T + 1 + t] = -(2.0 ** (t + 1))
        cons[:, NG * T + 1 + T + t] = -(2.0 ** t)

    in_maps = []
    for i in range(N_CORES):
        # teT: [NPAIR, 128, NK*2R] with [pair, p, k*512 + (sub*R+r)] = te[t, row, d]
        tei = te[:, i * R : (i + 1) * R, :]       # [T, R, DT]
        tei = tei.reshape(NPAIR, 2 * R, DT)       # [pair, sub*R+r, d]
        tei_p = np.zeros((NPAIR, 2 * R, KD), np.float32)
        tei_p[:, :, :DT] = tei
        teT = np.ascontiguousarray(
            tei_p.reshape(NPAIR, 2 * R, NK, 128)
            .transpose(0, 3, 2, 1)                # [pair, p, k, n]
            .reshape(NPAIR, 128, NK * 2 * R)
            .astype(bf)
        )
        # seT region of wall: [p, k*T*NB + t*NB+b] = se[t, batch, d]
        sei = se[:, i * NB : (i + 1) * NB, :]     # [T, NB, DS]
        seT = padk(np.ascontiguousarray(sei.reshape(T * NB, DS).T))  # [KD, T*NB]
        wallM_i = np.empty((128, WM_COLS), bf)
        wallM_i[:, O_WB : O_WB + NK * F] = wallM_wb
        wallM_i[:, O_SE : O_SE + NK * T * NB] = _swizzle_kmaj(seT, T * NB)
        in_maps.append(dict(teT=teT, wallA=wallA, wallM=wallM_i, wallB=wallB, cons=cons))
    return in_maps


def assemble_out(core_outs):
    """[N_CORES][L, R] -> [B, L, S]"""
    full = np.stack([np.asarray(o, np.float32) for o in core_outs], axis=0)
    full = full.reshape(N_CORES, L, NB, S).transpose(0, 2, 1, 3)
    return np.ascontiguousarray(full.reshape(B, L, S))


_NC_CACHE = {}


def get_nc(with_b1=False, state_dt=BF16):
    key = ("nc", str(state_dt))
    if key not in _NC_CACHE:
        last = None
        for _ in range(6):
            try:
                _NC_CACHE[key] = build_nc(state_dt=state_dt)
                break
            except Exception as e:  # rare scheduler-order race-detector trip
                last = e
        else:
            raise last
    return _NC_CACHE[key]


def run_sharded(in_maps, with_b1=False, trace=False, **kw):
    nc = get_nc(with_b1=with_b1)
    if not getattr(nc, "_waits_split", False):
        _split_excess_waits(nc)
        nc._waits_split = True
    res = run_bass_kernel_spmd(
        nc, in_maps, core_ids=list(range(N_CORES)), trace=trace, **kw
    )
    return res


def kernel(**inputs):
    in_maps = prep_in_maps(inputs)
    with_b1 = bool(np.any(np.asarray(inputs["b1"], np.float32)))
    res = run_sharded(in_maps, with_b1=with_b1)
    return assemble_out([res.results[i]["out"] for i in range(N_CORES)])



# revision 4
# speedup vs baseline: 1.0362x; 1.0362x over previous
"""Trainium2 Bass kernel for nn_MCQuantiles (ThreeCompNode SNN scan).

Strategy (8 NeuronCores, data-parallel over batch):
- Each core takes 8 batches x 32 samples = 256 rows of the B*S axis.
- Everything runs in "transposed space": feature dims on SBUF partitions,
  batch-rows on the free dim. All transposes/swizzles are done host-side for
  free; every DMA is a flat contiguous [128, X] block.
- The apical matmul (81% of FLOPs) runs in fp8e4 DoubleRow mode (K=256 per
  instruction, 2 MACs/cell/cycle). Wa is pre-scaled by 64 host-side to avoid
  e4m3 subnormals; the 1/64 descale folds into the DVE update constants.
  Margin analysis (sim_fp8.py): layer-2 membrane max ~0.35 vs threshold 0.5,
  so fp8 noise (~0.02) cannot flip any output spike.
- Membrane recurrences use 2^t-scaled state so each update is a single fused
  op; states are fused [128, 4R] tiles (4 feature groups side by side) so one
  DVE/GpSimd instruction covers all groups.
- Engine split: DVE does psum-sourced updates + thresholds, GpSimd does the
  SBUF-only adds/resets, ACT adds the per-partition c1 bias.
- W1/W2/basal matmuls stay bf16 (fp8 would make them LDWEIGHTS-bound).
- out accumulates in a persistent PSUM bank over all T, evicted once with
  scale 1/T + bias b2.
"""
import numpy as np
import ml_dtypes

import bass_rust
import concourse.bass as bass
import concourse.mybir as mybir
from concourse.bass_utils import run_bass_kernel_spmd
from concourse.tile import TileContext
from concourse.tile_rust import add_dep_helper

# ----- problem constants (hardcoded per contract) -----
T, B, S = 8, 64, 32
DS = DT = 3136
F = H = 512
L = 18
N_CORES = 8
NB = B // N_CORES              # 8 batches per core
R = NB * S                     # 256 rows per core
NPAIR = T // 2                 # 4 step pairs
NG = F // 128                  # 4 f-tiles (= h-tiles)
WSCALE = 64.0                  # host-side fp8 weight scale (2^6, exact)

# apical path: fp8 DoubleRow, k-tiles of 256 (3136 -> 3328 = 13*256)
NK2 = 13
KD2 = NK2 * 256
# basal path: bf16, k-tiles of 128 (3136 -> 3200 = 25*128)
NKB = 25
KDB = NKB * 128

# column offsets inside wallM [128, *] (bf16)
O_WB = 0                       # basal weights, NKB*F cols
O_SE = O_WB + NKB * F          # state embeddings, NKB*T*NB cols
WM_COLS = O_SE + NKB * T * NB
# wallB (bf16): W1.T then W2.T
O_W1 = 0
O_W2 = O_W1 + NG * H
WB_COLS = O_W2 + NG * L

F32 = mybir.dt.float32
BF16 = mybir.dt.bfloat16
F8 = mybir.dt.float8e4
OP = mybir.AluOpType
DR = mybir.MatmulPerfMode.DoubleRow

# apical DMA chunks, in k256 units (total NK2=13); chunk 0 small so the
# first matmul can start early
CHUNKS2 = [1, 2, 2, 2, 3, 3]
CH2_OFF = [0, 1, 3, 5, 7, 10]
NCHUNK = len(CHUNKS2)


def _patch_tile_drain():
    """This walrus build allows a single sync-wait per TPB_CTRL Drain; Tile's
    kernel-tail drain attaches one wait per active logical proc. Split them
    across a chain of drains."""
    def _patched(self, tick_clock, wait_clock):
        nc = self.nc
        drain_inst = nc.sync.drain()
        wait_clock.add_sem_waits(
            drain_inst.ins, bass_rust.ScopedClock({None: tick_clock.global_clock})
        )
        si = drain_inst.ins.sync_info
        if si is not None and len(si.on_wait) > 1:
            waits = list(si.on_wait)
            drain_inst.ins.sync_info = mybir.SyncInfo(
                on_wait=waits[:1], on_update=list(si.on_update)
            )
            for w in waits[1:]:
                extra = nc.sync.drain()
                extra.ins.sync_info = mybir.SyncInfo(on_wait=[w], on_update=[])
        nc.all_engine_barrier()
        popped = nc._tile_sem_poison_stack.pop()
        assert popped is self._sem_poison
        nc.clear_and_free_semaphores(list(self.sems.allocated().values()))
        nc.all_engine_barrier()

    TileContext._drain_and_barrier = _patched


def _split_excess_waits(nc, limit=1):
    """Walrus here rejects instructions carrying more than ~1 sync-wait. Move
    excess waits onto same-engine NoOps inserted just before the instruction."""
    for fn in nc.m.functions:
        for bb in fn.blocks:
            new = []
            changed = False
            for inst in bb.instructions:
                si = getattr(inst, "sync_info", None)
                ow = list(si.on_wait) if si is not None and si.on_wait else []
                if len(ow) > limit:
                    extra = ow[limit:]
                    for j in range(0, len(extra), limit):
                        nop = mybir.InstNoOp(
                            name=f"{inst.name}-ws{j}", ins=[], outs=[]
                        )
                        nop.engine = inst.engine
                        nop.sync_info = mybir.SyncInfo(
                            on_wait=extra[j : j + limit], on_update=[]
                        )
                        new.append(nop)
                    inst.sync_info = mybir.SyncInfo(
                        on_wait=ow[:limit], on_update=list(si.on_update)
                    )
                    changed = True
                new.append(inst)
            if changed:
                try:
                    bb.instructions[:] = new
                except TypeError:
                    bb.instructions = new


def build_nc():
    _patch_tile_drain()
    nc = bass.Bass()

    teT = nc.declare_dram_parameter("teT", [NPAIR, 128, NK2 * 1024], F8, isOutput=False)
    wallA = nc.declare_dram_parameter("wallA", [128, NK2 * 1024], F8, isOutput=False)
    wallM = nc.declare_dram_parameter("wallM", [128, WM_COLS], BF16, isOutput=False)
    wallB = nc.declare_dram_parameter("wallB", [128, WB_COLS], BF16, isOutput=False)
    cons = nc.declare_dram_parameter("cons", [128, NG * T + 1], F32, isOutput=False)
    out = nc.declare_dram_parameter("out", [L, R], F32, isOutput=True)

    with TileContext(nc) as tc:
        with (
            tc.tile_pool(name="wpool", bufs=1) as wpool,
            tc.tile_pool(name="tepool", bufs=2) as tepool,
            tc.tile_pool(name="state", bufs=1) as state,
            tc.tile_pool(name="qpool", bufs=2) as qpool,
            tc.tile_pool(name="appool", bufs=1, space="PSUM") as appool,
            tc.tile_pool(name="hqpool", bufs=1, space="PSUM") as hqpool,
            tc.tile_pool(name="bopool", bufs=1, space="PSUM") as bopool,
        ):
            # ---- resident weights/constants ----
            wallA_c = []
            prev = None
            for c in range(NCHUNK):
                ck = wpool.tile(
                    [128, CHUNKS2[c] * 1024], F8, tag=f"wallA{c}", name=f"wa_ck{c}"
                )
                wallA_c.append(ck)
                d = nc.sync.dma_start(
                    ck[:],
                    wallA[:, CH2_OFF[c] * 1024 : (CH2_OFF[c] + CHUNKS2[c]) * 1024],
                )
                if prev is not None:
                    add_dep_helper(d.ins, prev.ins, reason="serialize wallA chunks")
                prev = d
            wallM_sb = wpool.tile([128, WM_COLS], BF16, tag="wallM", name="wallM_sb")
            wallB_sb = wpool.tile([128, WB_COLS], BF16, tag="wallB", name="wallB_sb")
            cons_sb = wpool.tile([128, NG * T + 1], F32, tag="cons", name="cons_sb")

            def waT8(c, kk, g):
                """fp8 DoubleRow lhsT [128, 2, 128] for local k256 kk, group g."""
                base = kk * 1024 + g * 256
                return wallA_c[c][:, base : base + 256].rearrange(
                    "p (two f) -> p two f", two=2
                )

            def wbT(k, g):
                return wallM_sb[:, O_WB + k * F + g * 128 : O_WB + k * F + (g + 1) * 128]

            def seT(k):
                return wallM_sb[:, O_SE + k * T * NB : O_SE + (k + 1) * T * NB]

            def w1T(k, g):
                return wallB_sb[:, O_W1 + k * H + g * 128 : O_W1 + k * H + (g + 1) * 128]

            def w2T(k):
                return wallB_sb[:, O_W2 + k * L : O_W2 + (k + 1) * L]

            def c1s_ap(g, t):
                return cons_sb[:, g * T + t : g * T + t + 1]

            b2_ap = cons_sb[0:L, NG * T : NG * T + 1]

            # ---- fused state tiles ([128, 4R]: 4 groups side by side) ----
            A = [state.tile([128, NG * R], BF16, tag=f"A{p}", name=f"A{p}")
                 for p in range(2)]          # alpha ping-pong (even/odd t)
            M = state.tile([128, NG * R], BF16, tag="M", name="M")
            ML = state.tile([128, NG * R], BF16, tag="ML", name="ML")
            Bsc = state.tile([128, T * NG * NB], BF16, tag="Bsc", name="Bsc")

            o_psum = bopool.tile([L, R], F32, tag="o", name="o_psum")

            def A3(p):
                return A[p].rearrange("p (g r) -> p g r", g=NG)

            def M_gbs():
                return M.rearrange("p (gb s) -> p gb s", s=S)

            def A_gbs(p):
                return A[p].rearrange("p (gb s) -> p gb s", s=S)

            def B_bc(t):
                return (
                    Bsc[:, t * NG * NB : (t + 1) * NG * NB]
                    .unsqueeze(2)
                    .broadcast_to([128, NG * NB, S])
                )

            # ---- emission helpers ----
            def emit_te_dma(pair, chain):
                tiles = []
                prev = None
                for c in range(NCHUNK):
                    tck = tepool.tile(
                        [128, CHUNKS2[c] * 1024], F8, tag=f"te{c}", name=f"te_ck{c}"
                    )
                    tiles.append(tck)
                    d = nc.sync.dma_start(
                        tck[:],
                        teT[pair][:, CH2_OFF[c] * 1024
                                  : (CH2_OFF[c] + CHUNKS2[c]) * 1024],
                    )
                    if prev is not None and chain:
                        add_dep_helper(d.ins, prev.ins,
                                       reason="serialize te chunk DMAs")
                    prev = d
                return tiles, prev

            def emit_ap_chunk(mega, te_tiles, c):
                for g in range(NG):
                    for kk in range(CHUNKS2[c]):
                        k = CH2_OFF[c] + kk
                        rhs = te_tiles[c][:, kk * 1024 : (kk + 1) * 1024].rearrange(
                            "p (two n) -> p two n", two=2
                        )
                        nc.tensor.matmul(
                            mega[:, g * 512 : (g + 1) * 512],
                            lhsT=waT8(c, kk, g),
                            rhs=rhs,
                            start=(k == 0),
                            stop=(k == NK2 - 1),
                            perf_mode=DR,
                        )

            def emit_a_updates(mega, pair):
                """alpha_t = alpha_{t-1} + (2^{t-1}/64) * ap64_t, fused over
                groups; ping-pong A[t%2]."""
                mega3 = mega.rearrange("p (g x) -> p g x", g=NG)
                for sub in range(2):
                    t = 2 * pair + sub
                    s_t = float(2 ** (t - 1)) / WSCALE
                    ap3 = mega3[:, :, sub * R : (sub + 1) * R]
                    if t == 0:
                        nc.vector.tensor_scalar(A3(0), ap3, s_t, None, OP.mult)
                    else:
                        nc.vector.scalar_tensor_tensor(
                            A3(t % 2), ap3, s_t, A3(1 - t % 2), OP.mult, OP.add
                        )

            def emit_basal():
                bs_ps = bopool.tile([128, NG * T * NB], F32, tag="bs", name="bs_ps")
                for g in range(NG):
                    for k in range(NKB):
                        nc.tensor.matmul(
                            bs_ps[:, g * T * NB : (g + 1) * T * NB],
                            lhsT=wbT(k, g),
                            rhs=seT(k),
                            start=(k == 0),
                            stop=(k == NKB - 1),
                        )
                ps3 = bs_ps.rearrange("p (g tb) -> p g tb", g=NG)
                for t in range(T):
                    src = ps3[:, :, t * NB : (t + 1) * NB]
                    dst = Bsc[:, t * NG * NB : (t + 1) * NG * NB].rearrange(
                        "p (g b) -> p g b", g=NG
                    )
                    if t == 0:
                        nc.vector.tensor_scalar(dst, src, 0.5, None, OP.mult)
                    else:
                        prv = Bsc[:, (t - 1) * NG * NB : t * NG * NB].rearrange(
                            "p (g b) -> p g b", g=NG
                        )
                        nc.vector.scalar_tensor_tensor(
                            dst, src, float(2 ** (t - 1)), prv, OP.mult, OP.add
                        )

            def emit_sub(pair, sub):
                t = 2 * pair + sub
                th_t = float(2 ** (t + 1))
                sc_t = float(2 ** t)
                Ax = A[t % 2]
                # --- membrane: M = M_post + A_t + B_t (GpSimd, SBUF-only) ---
                if t == 0:
                    nc.gpsimd.tensor_tensor(M_gbs(), A_gbs(0), B_bc(0), OP.add)
                else:
                    nc.gpsimd.tensor_tensor(M[:], M[:], Ax[:], OP.add)
                    nc.gpsimd.tensor_tensor(M_gbs(), B_bc(t), M_gbs(), OP.add)
                # spikes (DVE): q = NOT(spike) = (M <= th)
                qg = qpool.tile([128, NG * R], BF16, tag="q", name="qg")
                nc.vector.tensor_scalar(qg[:], M[:], th_t, None, OP.is_le)
                # hard reset (GpSimd): M *= q  (q = NOT spike)
                nc.gpsimd.tensor_tensor(M[:], M[:], qg[:], OP.mult)

                # --- layer 1: hq = q @ W1.T (bf16), fused psum ---
                hq = hqpool.tile([128, NG * R], F32, tag="hq", name="hq")
                for g in range(NG):
                    for k in range(NG):
                        nc.tensor.matmul(
                            hq[:, g * R : (g + 1) * R],
                            lhsT=w1T(k, g),
                            rhs=qg[:, k * R : (k + 1) * R],
                            start=(k == 0),
                            stop=(k == NG - 1),
                        )

                # --- LIF: ML = ML - 2^t * hq  (+ 2^t*c1 via ACT) ---
                if t == 0:
                    nc.vector.tensor_scalar(ML[:], hq[:], -1.0, None, OP.mult)
                else:
                    nc.vector.scalar_tensor_tensor(
                        ML[:], hq[:], -sc_t, ML[:], OP.mult, OP.add
                    )
                for g in range(NG):
                    sl = ML[:, g * R : (g + 1) * R]
                    nc.scalar.activation(
                        sl, sl, mybir.ActivationFunctionType.Identity,
                        bias=c1s_ap(g, t), scale=1.0,
                    )
                spg = qpool.tile([128, NG * R], BF16, tag="sp2", name="spg")
                nc.vector.tensor_scalar(spg[:], ML[:], sc_t, None, OP.is_gt)
                qL = qpool.tile([128, NG * R], BF16, tag="qL", name="qL")
                nc.vector.tensor_scalar(qL[:], ML[:], sc_t, None, OP.is_le)
                nc.gpsimd.tensor_tensor(ML[:], ML[:], qL[:], OP.mult)

                # --- layer 2: out += sp2 @ W2.T (bf16, persistent psum) ---
                for k in range(NG):
                    nc.tensor.matmul(
                        o_psum[:],
                        lhsT=w2T(k),
                        rhs=spg[:, k * R : (k + 1) * R],
                        start=(t == 0 and k == 0),
                        stop=(t == T - 1 and k == NG - 1),
                    )

            # ---- prologue: pair 0 load + apical ----
            te_tiles, last_te_dma = emit_te_dma(0, chain=True)
            mega = appool.tile([128, NG * 512], F32, tag="ap", name="ap_mega")
            for c in range(NCHUNK):
                emit_ap_chunk(mega, te_tiles, c)
            dM = nc.sync.dma_start(wallM_sb[:], wallM[:])
            add_dep_helper(dM.ins, last_te_dma.ins, reason="wallM after te0 chain")
            dB = nc.sync.dma_start(wallB_sb[:], wallB[:])
            add_dep_helper(dB.ins, dM.ins, reason="wallB after wallM")
            dC = nc.sync.dma_start(cons_sb[:], cons[:])
            add_dep_helper(dC.ins, dM.ins, reason="cons after wallM")
            emit_basal()

            # ---- software-pipelined main loop ----
            for pair in range(NPAIR):
                emit_a_updates(mega, pair)
                if pair + 1 < NPAIR:
                    te_tiles, _ = emit_te_dma(pair + 1, chain=False)
                    meganxt = appool.tile([128, NG * 512], F32, tag="ap",
                                          name="ap_mega")
                    emit_ap_chunk(meganxt, te_tiles, 0)
                    emit_ap_chunk(meganxt, te_tiles, 1)
                    emit_sub(pair, 0)
                    emit_ap_chunk(meganxt, te_tiles, 2)
                    emit_ap_chunk(meganxt, te_tiles, 3)
                    emit_sub(pair, 1)
                    emit_ap_chunk(meganxt, te_tiles, 4)
                    emit_ap_chunk(meganxt, te_tiles, 5)
                    mega = meganxt
                else:
                    emit_sub(pair, 0)
                    emit_sub(pair, 1)

            # ---- final eviction: out = o_psum / T + b2 ----
            out_sb = state.tile([L, R], F32, tag="out_sb", name="out_sb")
            nc.scalar.activation(
                out_sb[:], o_psum[:],
                mybir.ActivationFunctionType.Identity,
                bias=b2_ap, scale=1.0 / T,
            )
            nc.sync.dma_start(out[:], out_sb[:])

    return nc


def _f8(a):
    """fp32 -> TRN fp8e4 (IEEE e4m3, max 240) with clip."""
    return np.clip(a, -240.0, 240.0).astype(ml_dtypes.float8_e4m3)


def prep_in_maps(inputs):
    """Host-side shard + transpose + pad + cast. Returns list of per-core dicts."""
    se = np.asarray(inputs["state_embedding"], np.float32)
    te = np.asarray(inputs["tau_embedding"], np.float32)
    Wb = np.asarray(inputs["Wb"], np.float32)
    Wa = np.asarray(inputs["Wa"], np.float32)
    W1 = np.asarray(inputs["W1"], np.float32)
    b1 = np.asarray(inputs["b1"], np.float32)
    W2 = np.asarray(inputs["W2"], np.float32)
    b2 = np.asarray(inputs["b2"], np.float32)
    bf = ml_dtypes.bfloat16

    def padk(a, kd):  # pad feature axis 0
        o = np.zeros((kd,) + a.shape[1:], a.dtype)
        o[: a.shape[0]] = a
        return o

    def swz(a, cols):  # [KDB, cols] -> [128, NKB*cols] bf16 k-major
        nk = a.shape[0] // 128
        return np.ascontiguousarray(
            a.reshape(nk, 128, cols).transpose(1, 0, 2).reshape(128, nk * cols)
            .astype(bf)
        )

    # wallA (fp8): col = k256*1024 + g*256 + kk*128 + m
    WaT64 = padk(np.ascontiguousarray(Wa.T) * WSCALE, KD2)       # [KD2, F]
    wallA = np.ascontiguousarray(
        WaT64.reshape(NK2, 2, 128, NG, 128)      # [k, kk, p, g, m]
        .transpose(2, 0, 3, 1, 4)                # [p, k, g, kk, m]
        .reshape(128, NK2 * 1024)
    )
    wallA = _f8(wallA)

    wallM_wb = swz(padk(np.ascontiguousarray(Wb.T), KDB), F)
    wallB = np.empty((128, WB_COLS), bf)
    wallB[:, O_W1 : O_W1 + NG * H] = swz(np.ascontiguousarray(W1.T), H)
    wallB[:, O_W2 : O_W2 + NG * L] = swz(np.ascontiguousarray(W2.T), L)

    cons = np.zeros((128, NG * T + 1), np.float32)
    c1 = W1.sum(axis=1) + b1
    for g in range(NG):
        for t in range(T):
            cons[:, g * T + t] = c1[g * 128 : (g + 1) * 128] * (2.0 ** t)
    cons[:L, NG * T] = b2

    in_maps = []
    for i in range(N_CORES):
        # teT fp8: [pair, p, k128*512 + sub*R + r] ; k128 = 2*k256+kk
        tei = te[:, i * R : (i + 1) * R, :]       # [T, R, DT]
        tei = tei.reshape(NPAIR, 2 * R, DT)
        tei_p = np.zeros((NPAIR, 2 * R, KD2), np.float32)
        tei_p[:, :, :DT] = tei
        teT = np.ascontiguousarray(
            tei_p.reshape(NPAIR, 2 * R, 2 * NK2, 128)
            .transpose(0, 3, 2, 1)                # [pair, p, k128, n]
            .reshape(NPAIR, 128, NK2 * 1024)
        )
        teT = _f8(teT)
        # seT region of wallM: [p, k*T*NB + t*NB+b] = se[t, batch, d]
        sei = se[:, i * NB : (i + 1) * NB, :]     # [T, NB, DS]
        seTt = padk(np.ascontiguousarray(sei.reshape(T * NB, DS).T), KDB)
        wallM_i = np.empty((128, WM_COLS), bf)
        wallM_i[:, O_WB : O_WB + NKB * F] = wallM_wb
        wallM_i[:, O_SE : O_SE + NKB * T * NB] = swz(seTt, T * NB)
        in_maps.append(dict(teT=teT, wallA=wallA, wallM=wallM_i,
                            wallB=wallB, cons=cons))
    return in_maps


def assemble_out(core_outs):
    """[N_CORES][L, R] -> [B, L, S]"""
    full = np.stack([np.asarray(o, np.float32) for o in core_outs], axis=0)
    full = full.reshape(N_CORES, L, NB, S).transpose(0, 2, 1, 3)
    return np.ascontiguousarray(full.reshape(B, L, S))


_NC_CACHE = {}


def get_nc():
    key = "nc"
    if key not in _NC_CACHE:
        last = None
        for _ in range(6):
            try:
                _NC_CACHE[key] = build_nc()
                break
            except Exception as e:  # rare scheduler-order race-detector trip
                last = e
        else:
            raise last
    return _NC_CACHE[key]


def run_sharded(in_maps, trace=False, **kw):
    nc = get_nc()
    if not getattr(nc, "_waits_split", False):
        _split_excess_waits(nc)
        nc._waits_split = True
    res = run_bass_kernel_spmd(
        nc, in_maps, core_ids=list(range(N_CORES)), trace=trace, **kw
    )
    return res


def kernel(**inputs):
    in_maps = prep_in_maps(inputs)
    res = run_sharded(in_maps)
    return assemble_out([res.results[i]["out"] for i in range(N_CORES)])
